# revision 1
# baseline (speedup 1.0000x reference)
"""Multi-head self-attention (B=8, S=2048, H=256, NH=8, HD=32) on 8 TRN2 cores.

Strategy: data-parallel over batch — each core computes full MHA for one
batch element; no collectives.

Per-core dataflow (all matmuls bf16 in / fp32 PSUM accum):
  - host ships x^T (features on partitions) so no on-device transpose
  - qkT:  q^T,k^T [feat, s] = w_qkv^T @ x — feature-major so each head's
    32 q/k features land on one 32-partition strip
  - scores^T per (head, key-tile): 4 heads computed concurrently via
    4x row-tiled PE (tile_position=(32i,0), K=32)
  - exp on ScalarE straight out of a 4-bank PSUM region ([128,2048] per
    ACTIVATE, scale=1/sqrt(HD) folded in); softmax max-subtraction is
    skipped (scores are O(1), no overflow risk in fp32)
  - ctx^T accumulated over key tiles with 2x column-tiled PE
    (tile_position=(0,0)/(0,64)); stationary v blocks carry a ones
    column so each 64-row tile yields [ctx_h(32) | rowsum(1) | pad]
  - ctx evicted unnormalized to SBUF staging (frees the accumulators for
    the next q-block); the 8 rowsums per q-block are gathered via DRAM,
    one batched VectorE reciprocal, partition-broadcast back via DRAM,
    and multiplied in from staging
  - out = ctxT^T @ w_out_perm + b_out; w_out rows are permuted/zero-padded
    on the host to match the ctxT slot layout
"""
import numpy as np
import ml_dtypes

import bass_rust
import concourse.bass as bass
import concourse.mybir as mybir
import concourse.tile as tile
from concourse.vector_clock import ScopedClock
from concourse.bass_utils import run_bass_kernel_spmd

BF16 = mybir.dt.bfloat16
F32 = mybir.dt.float32
NPBF16 = ml_dtypes.bfloat16

B, S, H = 8, 2048, 256
NH, HD = 8, 32
SCALE = 1.0 / float(np.sqrt(HD))
N_CORES = 8

# Set by a test harness to collect HW timing: {"trace": bool, "trace_cores": [...]}
TRACE_OPTS = {}
LAST_RESULT = None

def _legalize_sync_waits(nc):
    """The walrus build here rejects >1 sync wait per instruction, but Tile
    freely emits 2-3 (and the exit drain up to ~27).  Move excess waits onto
    same-engine NoOp carriers inserted immediately before the offending
    instruction — identical semantics (the engine blocks on each wait in
    program order)."""
    n = 0
    for f in nc.m.functions:
        for bb in f.blocks:
            insts = bb.instructions  # live list
            i = 0
            while i < len(insts):
                inst = insts[i]
                si = inst.sync_info
                if si is not None and len(si.on_wait) > 1:
                    waits = list(si.on_wait)
                    carriers = []
                    for w in waits[:-1]:
                        carriers.append(
                            mybir.InstNoOp(
                                name=f"{inst.name}-w{n}",
                                sync_info=mybir.SyncInfo(on_wait=[w], on_update=[]),
                                bass_nofuse=True,
                                engine=inst.engine,
                            )
                        )
                        n += 1
                    inst.sync_info = bass_rust.SyncInfo(
                        on_wait=waits[-1:], on_update=list(si.on_update)
                    )
                    insts[i:i] = carriers
                    i += len(carriers)
                i += 1
    return n


def _build_nc(legalize=True):
    nc = bass.Bass()
    xt = nc.dram_tensor("xt", [128, 2 * S], BF16, kind="ExternalInput")
    wqk = nc.dram_tensor("wqk", [128, 2 * 512], BF16, kind="ExternalInput")
    bv = nc.dram_tensor("bv", [1, 264], BF16, kind="ExternalInput")
    wv = nc.dram_tensor("wv", [128, 2 * 264], BF16, kind="ExternalInput")
    wo = nc.dram_tensor("wo", [128, 4 * 256], BF16, kind="ExternalInput")
    bqkc = nc.dram_tensor("bqkc", [128, 4], F32, kind="ExternalInput")
    ones = nc.dram_tensor("ones", [1, 512], BF16, kind="ExternalInput")
    zrow = nc.dram_tensor("zrow", [2, 2048], BF16, kind="ExternalInput")
    out = nc.dram_tensor("out", [S, H], F32, kind="ExternalOutput")
    # scratch for the partition-broadcast DMA roundtrip (SBUF APs cannot
    # have a zero partition step, DRAM APs can): one row per (qb, pair, side)
    rscr = nc.dram_tensor("rscr", [32, 512], F32)
    rscr2 = nc.dram_tensor("rscr2", [32, 512], F32)

    EXP = mybir.ActivationFunctionType.Exp

    with tile.TileContext(nc) as tc:
        with (
            tc.tile_pool(name="const", bufs=1) as const,
            tc.tile_pool(name="ev", bufs=8) as ev,
            tc.tile_pool(name="etp", bufs=4) as etp,
        ):
            xt_sb = const.tile([128, 2 * S], BF16, tag="xt")
            nc.sync.dma_start(out=xt_sb, in_=xt[:, :])
            wqk_sb = const.tile([128, 2 * 512], BF16, tag="wqk")
            nc.sync.dma_start(out=wqk_sb, in_=wqk[:, :])
            wv_sb = const.tile([128, 2 * 264], BF16, tag="wv")
            nc.sync.dma_start(out=wv_sb, in_=wv[:, :])
            wo_sb = const.tile([128, 4 * 256], BF16, tag="wo")
            nc.sync.dma_start(out=wo_sb, in_=wo[:, :])
            bv_sb = const.tile([1, 264], BF16, tag="bv")
            nc.sync.dma_start(out=bv_sb, in_=bv[:, :])
            ones1_sb = const.tile([1, 128], BF16, tag="ones1")
            nc.sync.dma_start(out=ones1_sb, in_=ones[0:1, 0:128])
            bqkc_sb = const.tile([128, 4], F32, tag="bqkc")
            nc.sync.dma_start(out=bqkc_sb, in_=bqkc[:, :])

            qT_sb = const.tile([128, 2 * S], BF16, tag="qT")
            kT_sb = const.tile([128, 2 * S], BF16, tag="kT")
            v_sb = const.tile([128, 16 * 264], BF16, tag="v")
            ctxT_sb = [
                const.tile([128, S], BF16, tag=f"ctxT{k}", name=f"ctxT{k}")
                for k in range(4)
            ]
            # rows 32:64 / 96:128 of each ctxT tile are never written by the
            # evictions but are contracted by the output matmul (against
            # zeroed w_out rows) — clear them via broadcast DMA so stale NaN
            # patterns can't poison the accumulation
            for k in range(4):
                if k == 0:
                    # row 32 of tile 0 is all-ones: paired with w_out_perm
                    # row 32 = b_out it adds the output bias for free
                    nc.sync.dma_start(out=ctxT_sb[0][32:33, :], in_=zrow[1:2, :])
                    nc.sync.dma_start(
                        out=ctxT_sb[0][33:64, :],
                        in_=zrow[0:1, :].to_broadcast((31, S)),
                    )
                else:
                    nc.sync.dma_start(
                        out=ctxT_sb[k][32:64, :],
                        in_=zrow[0:1, :].to_broadcast((32, S)),
                    )
                nc.sync.dma_start(
                    out=ctxT_sb[k][96:128, :],
                    in_=zrow[0:1, :].to_broadcast((32, S)),
                )

            # ---- phase 0: HAM warmup — ~6µs of dep-free back-to-back
            # matmuls so the PE clock gate opens (1.2 -> 2.4 GHz) before the
            # real work; garbage values land in a scratch PSUM bank that is
            # never read ----
            with tc.tile_pool(name="pp", bufs=4, space="PSUM") as pp:
                warm_sb = const.tile([128, 512], BF16, tag="warm")
                nc.vector.memset(warm_sb, 0.0)
                warm_ps = pp.tile([128, 512], F32, tag="pp")
                for _ in range(12):
                    nc.tensor.matmul(
                        out=warm_ps, lhsT=warm_sb[:, 0:128], rhs=warm_sb[:, :],
                        start=True, stop=True,
                    )

                # ---- phase 1: qT/kT [feature, s] = w_qkv^T @ x; bias folded
                #      into the eviction (per-partition, features-major) ----
                for t in range(4):  # feature tiles: q0,q1,k0,k1
                    for nb in range(4):  # s blocks of 512
                        ps = pp.tile([128, 512], F32, tag="pp")
                        for ks in range(2):
                            nc.tensor.matmul(
                                out=ps,
                                lhsT=wqk_sb[:, ks * 512 + t * 128 : ks * 512 + t * 128 + 128],
                                rhs=xt_sb[:, ks * S + nb * 512 : ks * S + nb * 512 + 512],
                                start=(ks == 0), stop=(ks == 1),
                            )
                        dst = (qT_sb if t < 2 else kT_sb)[
                            :, (t % 2) * S + nb * 512 : (t % 2) * S + nb * 512 + 512
                        ]
                        nc.vector.tensor_scalar_add(
                            out=dst, in0=ps, scalar1=bqkc_sb[:, t : t + 1]
                        )

                # ---- phase 2: v (natural layout, padded 64-wide head slots,
                #      ones column at j=32 for rowsums) ----
                for st in range(16):
                    ps = pp.tile([128, 264], F32, tag="ppv")
                    for ks in range(2):
                        nc.tensor.matmul(
                            out=ps,
                            lhsT=xt_sb[:, ks * S + st * 128 : ks * S + st * 128 + 128],
                            rhs=wv_sb[:, ks * 264 : ks * 264 + 264],
                            start=(ks == 0), stop=False,
                        )
                    # bias row also plants the rowsum ones-columns
                    nc.tensor.matmul(
                        out=ps,
                        lhsT=ones1_sb[0:1, 0:128],
                        rhs=bv_sb[0:1, 0:264],
                        start=False, stop=True,
                    )
                    dst = v_sb[:, st * 264 : st * 264 + 264]
                    nc.scalar.copy(out=dst, in_=ps)

            # ---- phase 3: attention, q-blocks of 512 ----
            with (
                tc.tile_pool(name="scp", bufs=2, space="PSUM") as scp,
                tc.tile_pool(name="cxp", bufs=4, space="PSUM") as cxp,
            ):
                for qb in range(4):
                    ctx_tiles = [
                        cxp.tile([128, 512], F32, tag="ctx", name=f"ctx_{qb}_{p}")
                        for p in range(4)
                    ]

                    def emit_ctx(g, kt, eT):
                        # ctx accumulation for (g, kt) — emitted one
                        # iteration late so these PE matmuls fill the window
                        # while ACT runs the *next* exp
                        for pi in range(2):
                            pair = g * 2 + pi
                            cps = ctx_tiles[pair]
                            vc = kt * 264 + pair * 66
                            nc.tensor.matmul(
                                out=cps[0:33, :],
                                lhsT=v_sb[:, vc : vc + 33],
                                rhs=eT[:, (2 * pi) * 512 : (2 * pi) * 512 + 512],
                                start=(kt == 0), stop=(kt == 15),
                                tile_position=(0, 0), skip_group_check=True,
                            )
                            nc.tensor.matmul(
                                out=cps[64:97, :],
                                lhsT=v_sb[:, vc + 33 : vc + 66],
                                rhs=eT[:, (2 * pi + 1) * 512 : (2 * pi + 1) * 512 + 512],
                                start=(kt == 0), stop=(kt == 15),
                                tile_position=(0, 64), skip_group_check=True,
                            )

                    pending = None
                    for kt in range(16):
                        for g in range(2):  # head groups of 4
                            eT = etp.tile([128, 2048], BF16, tag="eT")
                            # two half-groups in separate PSUM tiles: the
                            # half-B exp's WAR doesn't block half-A scores,
                            # so the next scores always overlap the running
                            # exp and ACT never waits on the PE
                            for half in range(2):
                                sc = scp.tile([128, 1024], F32, tag="sc",
                                              name=f"sc_{qb}_{kt}_{g}_{half}")
                                for i in (2 * half, 2 * half + 1):
                                    nc.tensor.matmul(
                                        out=sc[:, (i % 2) * 512 : (i % 2) * 512 + 512],
                                        lhsT=kT_sb[32 * i : 32 * i + 32,
                                                   g * S + kt * 128 : g * S + kt * 128 + 128],
                                        rhs=qT_sb[32 * i : 32 * i + 32,
                                                  g * S + qb * 512 : g * S + qb * 512 + 512],
                                        start=True, stop=True,
                                        tile_position=(32 * i, 0),
                                    )
                                nc.scalar.activation(
                                    out=eT[:, half * 1024 : half * 1024 + 1024],
                                    in_=sc,
                                    func=EXP, scale=SCALE,
                                )
                            if pending is not None:
                                emit_ctx(*pending)
                            pending = (g, kt, eT)
                    emit_ctx(*pending)
                    # Evict unnormalized ctx PSUM -> SBUF staging right away so
                    # the accumulator banks free for the next q-block, then
                    # normalize off the critical path: gather the 8 rowsum
                    # rows via DRAM into one [8,512] tile, one batched
                    # reciprocal (cost ~ free size only), broadcast back
                    # across partitions via DRAM, multiply from staging.
                    stages = []
                    for pair in range(4):
                        stg = ev.tile([128, 512], F32, tag="stg",
                                      name=f"stg_{qb}_{pair}")
                        nc.vector.tensor_copy(
                            out=stg[0:33, :], in_=ctx_tiles[pair][0:33, :]
                        )
                        nc.vector.tensor_copy(
                            out=stg[64:97, :], in_=ctx_tiles[pair][64:97, :]
                        )
                        stages.append(stg)
                        r0 = (qb * 4 + pair) * 2
                        nc.sync.dma_start(out=rscr[r0 : r0 + 1, :], in_=stg[32:33, :])
                        nc.sync.dma_start(out=rscr[r0 + 1 : r0 + 2, :], in_=stg[96:97, :])
                    rsg = ev.tile([8, 512], F32, tag="rsg")
                    nc.sync.dma_start(out=rsg, in_=rscr[qb * 8 : qb * 8 + 8, :])
                    nc.vector.reciprocal(out=rsg, in_=rsg)
                    nc.sync.dma_start(out=rscr2[qb * 8 : qb * 8 + 8, :], in_=rsg)
                    for pair in range(4):
                        stg = stages[pair]
                        rcb = ev.tile([128, 512], F32, tag="rcb",
                                      name=f"rcb_{qb}_{pair}")
                        r0 = (qb * 4 + pair) * 2
                        nc.sync.dma_start(
                            out=rcb[0:32, :],
                            in_=rscr2[r0 : r0 + 1, :].to_broadcast((32, 512)),
                        )
                        nc.sync.dma_start(
                            out=rcb[64:96, :],
                            in_=rscr2[r0 + 1 : r0 + 2, :].to_broadcast((32, 512)),
                        )
                        dst = ctxT_sb[pair]
                        nc.vector.tensor_mul(
                            out=dst[0:32, qb * 512 : qb * 512 + 512],
                            in0=stg[0:32, :], in1=rcb[0:32, :],
                        )
                        nc.vector.tensor_mul(
                            out=dst[64:96, qb * 512 : qb * 512 + 512],
                            in0=stg[64:96, :], in1=rcb[64:96, :],
                        )

            # ---- phase 4: out = ctxT^T @ w_out_perm + b_out ----
            with tc.tile_pool(name="op", bufs=4, space="PSUM") as op:
                for st in range(16):
                    ps = op.tile([128, 256], F32, tag="op")
                    for kk in range(4):
                        nc.tensor.matmul(
                            out=ps,
                            lhsT=ctxT_sb[kk][:, st * 128 : st * 128 + 128],
                            rhs=wo_sb[:, kk * 256 : kk * 256 + 256],
                            start=(kk == 0), stop=(kk == 3),
                        )
                    ot = ev.tile([128, 256], F32, tag="ot")
                    nc.vector.tensor_copy(out=ot, in_=ps)
                    nc.sync.dma_start(
                        out=out[st * 128 : st * 128 + 128, :], in_=ot
                    )
    if legalize:
        _legalize_sync_waits(nc)
    return nc


_NC_CACHE = None


def _get_nc():
    global _NC_CACHE
    if _NC_CACHE is None:
        _NC_CACHE = _build_nc()
    return _NC_CACHE


def _ks_layout(a, nk, cols):
    """[nk*128, cols] -> [128, nk*cols] with [p, k*cols+c] = a[k*128+p, c]."""
    return np.ascontiguousarray(
        a.reshape(nk, 128, cols).transpose(1, 0, 2).reshape(128, nk * cols)
    )


def _prep_in_maps(x, w_qkv, b_qkv, w_out, b_out):
    x = np.asarray(x, dtype=np.float32)
    w_qkv = np.asarray(w_qkv, dtype=np.float32)
    b_qkv = np.asarray(b_qkv, dtype=np.float32)
    w_out = np.asarray(w_out, dtype=np.float32)
    b_out = np.asarray(b_out, dtype=np.float32)

    # shared (per-core identical) weight layouts
    wqk_l = _ks_layout(w_qkv[:, : 2 * H], 2, 512).astype(NPBF16)

    # v weights: 64-wide slot per head: [v_h (32) | ones-col | 31 zero]
    # (the ones column itself is DMA'd on device; v bias is zero per spec)
    wpad = np.zeros((H, 264), np.float32)
    bvr = np.zeros((1, 264), np.float32)
    for h in range(NH):
        c0 = h * 33
        wpad[:, c0 : c0 + 32] = w_qkv[:, 2 * H + h * HD : 2 * H + (h + 1) * HD]
        bvr[0, c0 : c0 + 32] = b_qkv[2 * H + h * HD : 2 * H + (h + 1) * HD]
        bvr[0, c0 + 32] = 1.0  # ones column -> rowsum row
    wv_l = _ks_layout(wpad, 2, 264).astype(NPBF16)


    # w_out rows permuted into the ctxT slot layout (zeros in pad slots)
    wo_perm = np.zeros((512, H), np.float32)
    for pair in range(4):
        for side in range(2):
            h = 2 * pair + side
            r0 = pair * 128 + side * 64
            wo_perm[r0 : r0 + 32, :] = w_out[h * HD : (h + 1) * HD, :]
    wo_perm[32, :] = b_out  # multiplied by the ctxT[0] ones row
    wo_l = _ks_layout(wo_perm, 4, 256).astype(NPBF16)

    shared = {
        "wqk": wqk_l,
        "wv": wv_l,
        "bv": bvr.astype(NPBF16),
        "wo": wo_l,
        "bqkc": np.ascontiguousarray(
            b_qkv[: 2 * H].astype(np.float32).reshape(4, 128).T
        ),
        "ones": np.ones((1, 512), NPBF16),
        "zrow": np.concatenate([np.zeros((1, 2048), NPBF16), np.ones((1, 2048), NPBF16)]),
    }
    in_maps = []
    for b in range(B):
        xt = _ks_layout(np.ascontiguousarray(x[b].T), 2, S).astype(NPBF16)
        in_maps.append({"xt": xt, **shared})
    return in_maps


def kernel(x, w_qkv, b_qkv, w_out, b_out):
    in_maps = _prep_in_maps(x, w_qkv, b_qkv, w_out, b_out)
    nc = _get_nc()
    res = run_bass_kernel_spmd(nc, in_maps, list(range(N_CORES)), **TRACE_OPTS)
    global LAST_RESULT
    LAST_RESULT = res
    return np.stack([res.results[b]["out"] for b in range(B)], axis=0)



# revision 7
# speedup vs baseline: 1.4430x; 1.4430x over previous
"""Multi-head self-attention (B=8, S=2048, H=256, NH=8, HD=32) on 8 TRN2 cores.

Strategy: data-parallel over batch — each core computes full MHA for one
batch element; no collectives.

Per-core dataflow (matmuls bf16 in / fp32 PSUM accum):
  - host ships x^T (features on partitions) so no on-device transpose
  - qkT:  q^T,k^T [feat, s] = w_qkv^T @ x — feature-major so each head's
    32 q/k features land on one 32-partition strip
  - attention runs qb(4) x g(2 head-groups) x kt(16) x half(2):
    per step 2 row-tiled scores matmuls (tile_position=(32j,0), K=32)
    into a 3-deep rotation of [128,1024] PSUM tiles, then ONE exp:
      - ACT steps: scalar ACTIVATE Exp (scale folded in)
      - DVE steps: Schraudolph bf16 exp — tensor_scalar mult+add to an
        int16 view of the bf16 eT tile (i16 = rne(s*A + B) IS the bf16
        bit pattern of ~exp(s*scale)); rowsum-normalization cancels the
        systematic part of the approx error per head
    Splitting exp across both engines breaks the single-engine ACT
    bottleneck (~331us of EXP in the previous version).
  - ctx^T accumulated over kt with 2x column-tiled PE (tile_position
    (0,0)/(0,64)); stationary v blocks carry a ones column so each
    64-wide head slot yields [ctx_h(32) | rowsum(1)]; ctx for kt-1 is
    emitted as a 4-matmul col-mode burst after kt's scores so row/col
    mode switches happen 2x per kt, mostly hidden behind exp waits
  - g-outer keeps only 2 ctx accumulator banks live -> scores get a
    3-tile rotation (6 banks) which decouples the scores->exp WAR chain
  - per (qb,g): ctx PSUM evicted to bf16 staging in one [97,512] copy;
    rowsum rows DMA'd straight from PSUM (fp32) to DRAM; per qb one
    batched fast-reciprocal + bf16 broadcast roundtrip via DRAM; the
    normalization multiplies (bf16, 2x/4x DVE modes) are emitted
    deferred (during the next qb) so DMA latency never stalls the DVE
  - out = ctxT^T @ w_out_perm + b_out as a tail phase; out-proj PSUM is
    DMA'd directly to DRAM (no SBUF staging)
"""
import numpy as np
import ml_dtypes

import bass_rust
import concourse.bass as bass
import concourse.mybir as mybir
import concourse.tile as tile
from concourse.bass_utils import run_bass_kernel_spmd

BF16 = mybir.dt.bfloat16
F32 = mybir.dt.float32
I16 = mybir.dt.int16
NPBF16 = ml_dtypes.bfloat16

B, S, H = 8, 2048, 256
NH, HD = 8, 32
SCALE = 1.0 / float(np.sqrt(HD))
N_CORES = 8

# Schraudolph bf16 exp constants: i16 = rne(s*A + B) viewed as bf16
# approximates exp(s*SCALE).  A = SCALE * 2^7 / ln2; B = 127*2^7 - c with
# c=1.5 calibrated for min global error (rne rounding confirmed on HW).
EXP_A = SCALE * 128.0 / float(np.log(2.0))
EXP_B = 16256.0 - 1.5

# ACT/DVE exp split: pattern over 32 steps, True -> ACT. 17/32 on ACT.
N_ACT_OF_32 = 17
ACT_PAT = [((i + 1) * N_ACT_OF_32) // 32 - (i * N_ACT_OF_32) // 32 == 1
           for i in range(32)]

# Set by a test harness to collect HW timing: {"trace": bool, "trace_cores": [...]}
TRACE_OPTS = {}
LAST_RESULT = None


def _legalize_sync_waits(nc):
    """The walrus build here rejects >1 sync wait per instruction, but Tile
    freely emits 2-3 (and the exit drain up to ~27).  Move excess waits onto
    same-engine NoOp carriers inserted immediately before the offending
    instruction — identical semantics (the engine blocks on each wait in
    program order)."""
    n = 0
    for f in nc.m.functions:
        for bb in f.blocks:
            insts = bb.instructions  # live list
            i = 0
            while i < len(insts):
                inst = insts[i]
                si = inst.sync_info
                if si is not None and len(si.on_wait) > 1:
                    waits = list(si.on_wait)
                    carriers = []
                    for w in waits[:-1]:
                        carriers.append(
                            mybir.InstNoOp(
                                name=f"{inst.name}-w{n}",
                                sync_info=mybir.SyncInfo(on_wait=[w], on_update=[]),
                                bass_nofuse=True,
                                engine=inst.engine,
                            )
                        )
                        n += 1
                    inst.sync_info = bass_rust.SyncInfo(
                        on_wait=waits[-1:], on_update=list(si.on_update)
                    )
                    insts[i:i] = carriers
                    i += len(carriers)
                i += 1
    return n


def _build_nc(legalize=True):
    nc = bass.Bass()
    xt = nc.dram_tensor("xt", [128, 2 * S], BF16, kind="ExternalInput")
    wqk = nc.dram_tensor("wqk", [128, 2 * 512], BF16, kind="ExternalInput")
    bv = nc.dram_tensor("bv", [1, 264], BF16, kind="ExternalInput")
    wv = nc.dram_tensor("wv", [128, 2 * 264], BF16, kind="ExternalInput")
    wo = nc.dram_tensor("wo", [128, 4 * 256], BF16, kind="ExternalInput")
    bqkc = nc.dram_tensor("bqkc", [128, 4], F32, kind="ExternalInput")
    ones = nc.dram_tensor("ones", [1, 512], BF16, kind="ExternalInput")
    zrow = nc.dram_tensor("zrow", [2, 2048], BF16, kind="ExternalInput")
    out = nc.dram_tensor("out", [S, H], F32, kind="ExternalOutput")
    # scratch for the rowsum reciprocal roundtrip (SBUF APs cannot have a
    # zero partition step, DRAM APs can): one row per (qb, g, h, side)
    rscr = nc.dram_tensor("rscr", [32, 512], BF16)
    rscr2 = nc.dram_tensor("rscr2", [32, 512], BF16)

    EXP = mybir.ActivationFunctionType.Exp

    with tile.TileContext(nc) as tc:
        with (
            tc.tile_pool(name="const", bufs=1) as const,
            tc.tile_pool(name="ev", bufs=4) as ev,
            tc.tile_pool(name="stgp", bufs=4) as stgp,
            tc.tile_pool(name="rcbp", bufs=4) as rcbp,
            tc.tile_pool(name="etp", bufs=6) as etp,
        ):
            xt_sb = const.tile([128, 2 * S], BF16, tag="xt")
            nc.sync.dma_start(out=xt_sb, in_=xt[:, :])
            wqk_sb = const.tile([128, 2 * 512], BF16, tag="wqk")
            nc.sync.dma_start(out=wqk_sb, in_=wqk[:, :])
            wv_sb = const.tile([128, 2 * 264], BF16, tag="wv")
            nc.sync.dma_start(out=wv_sb, in_=wv[:, :])
            wo_sb = const.tile([128, 4 * 256], BF16, tag="wo")
            nc.sync.dma_start(out=wo_sb, in_=wo[:, :])
            bv_sb = const.tile([1, 264], BF16, tag="bv")
            nc.sync.dma_start(out=bv_sb, in_=bv[:, :])
            ones1_sb = const.tile([1, 128], BF16, tag="ones1")
            nc.sync.dma_start(out=ones1_sb, in_=ones[0:1, 0:128])
            bqkc_sb = const.tile([128, 4], F32, tag="bqkc")
            nc.sync.dma_start(out=bqkc_sb, in_=bqkc[:, :])

            qT_sb = const.tile([128, 2 * S], BF16, tag="qT")
            kT_sb = const.tile([128, 2 * S], BF16, tag="kT")
            v_sb = const.tile([128, 16 * 264], BF16, tag="v")
            ctxT_sb = [
                const.tile([128, S], BF16, tag=f"ctxT{k}", name=f"ctxT{k}")
                for k in range(4)
            ]
            # rows 32:64 / 96:128 of each ctxT tile are never written by the
            # normalization muls but are contracted by the output matmul
            # (against zeroed w_out rows) — clear them via broadcast DMA so
            # stale NaN patterns can't poison the accumulation
            for k in range(4):
                if k == 0:
                    # row 32 of tile 0 is all-ones: paired with w_out_perm
                    # row 32 = b_out it adds the output bias for free
                    nc.sync.dma_start(out=ctxT_sb[0][32:33, :], in_=zrow[1:2, :])
                    nc.sync.dma_start(
                        out=ctxT_sb[0][33:64, :],
                        in_=zrow[0:1, :].to_broadcast((31, S)),
                    )
                else:
                    nc.sync.dma_start(
                        out=ctxT_sb[k][32:64, :],
                        in_=zrow[0:1, :].to_broadcast((32, S)),
                    )
                nc.sync.dma_start(
                    out=ctxT_sb[k][96:128, :],
                    in_=zrow[0:1, :].to_broadcast((32, S)),
                )

            # ---- phase 0: HAM warmup — dep-free back-to-back matmuls so the
            # PE clock gate opens (1.2 -> 2.4 GHz) before the real work ----
            with tc.tile_pool(name="pp", bufs=4, space="PSUM") as pp:
                warm_sb = const.tile([128, 512], BF16, tag="warm")
                nc.vector.memset(warm_sb, 0.0)
                warm_ps = pp.tile([128, 512], F32, tag="pp")
                for _ in range(12):
                    nc.tensor.matmul(
                        out=warm_ps, lhsT=warm_sb[:, 0:128], rhs=warm_sb[:, :],
                        start=True, stop=True,
                    )

                # ---- phase 1: qT/kT [feature, s] = w_qkv^T @ x; bias folded
                #      into the eviction (per-partition, features-major) ----
                for t in range(4):  # feature tiles: q0,q1,k0,k1
                    for nb in range(4):  # s blocks of 512
                        ps = pp.tile([128, 512], F32, tag="pp")
                        for ks in range(2):
                            nc.tensor.matmul(
                                out=ps,
                                lhsT=wqk_sb[:, ks * 512 + t * 128 : ks * 512 + t * 128 + 128],
                                rhs=xt_sb[:, ks * S + nb * 512 : ks * S + nb * 512 + 512],
                                start=(ks == 0), stop=(ks == 1),
                            )
                        dst = (qT_sb if t < 2 else kT_sb)[
                            :, (t % 2) * S + nb * 512 : (t % 2) * S + nb * 512 + 512
                        ]
                        nc.vector.tensor_scalar_add(
                            out=dst, in0=ps, scalar1=bqkc_sb[:, t : t + 1]
                        )

                # ---- phase 2: v (natural layout, padded 64-wide head slots,
                #      ones column at j=32 for rowsums) ----
                for st in range(16):
                    ps = pp.tile([128, 264], F32, tag="ppv")
                    for ks in range(2):
                        nc.tensor.matmul(
                            out=ps,
                            lhsT=xt_sb[:, ks * S + st * 128 : ks * S + st * 128 + 128],
                            rhs=wv_sb[:, ks * 264 : ks * 264 + 264],
                            start=(ks == 0), stop=False,
                        )
                    # bias row also plants the rowsum ones-columns
                    nc.tensor.matmul(
                        out=ps,
                        lhsT=ones1_sb[0:1, 0:128],
                        rhs=bv_sb[0:1, 0:264],
                        start=False, stop=True,
                    )
                    dst = v_sb[:, st * 264 : st * 264 + 264]
                    nc.scalar.copy(out=dst, in_=ps)

            # ---- phase 3: attention ----
            stg_tiles = {}   # (qb, g, h) -> stg tile

            with (
                tc.tile_pool(name="scp", bufs=3, space="PSUM") as scp,
                tc.tile_pool(name="cxp", bufs=2, space="PSUM") as cxp,
            ):
                def emit_ctx(qb, g, kt, ctx_t, eTs):
                    # ctx accumulation for (g, kt): 4 col-mode matmuls
                    for h in range(2):
                        cps = ctx_t[h]
                        eT = eTs[h]
                        vc = kt * 264 + (4 * g + 2 * h) * 33
                        nc.tensor.matmul(
                            out=cps[0:33, :],
                            lhsT=v_sb[:, vc : vc + 33],
                            rhs=eT[:, 0:512],
                            start=(kt == 0), stop=(kt == 15),
                            tile_position=(0, 0), skip_group_check=True,
                        )
                        nc.tensor.matmul(
                            out=cps[64:97, :],
                            lhsT=v_sb[:, vc + 33 : vc + 66],
                            rhs=eT[:, 512:1024],
                            start=(kt == 0), stop=(kt == 15),
                            tile_position=(0, 64), skip_group_check=True,
                        )

                def emit_recip(qb):
                    # batched rowsum reciprocal for all 8 (g,h,side) rows
                    rsg = ev.tile([8, 512], BF16, tag="rsg")
                    nc.sync.dma_start(out=rsg, in_=rscr[qb * 8 : qb * 8 + 8, :])
                    rsgf = ev.tile([8, 512], F32, tag="rsgf")
                    nc.vector.tensor_copy(out=rsgf, in_=rsg)
                    rsr = ev.tile([8, 512], F32, tag="rsr")
                    nc.vector.reciprocal(out=rsr, in_=rsgf)
                    rsb = ev.tile([8, 512], BF16, tag="rsb")
                    nc.vector.tensor_copy(out=rsb, in_=rsr)
                    nc.sync.dma_start(out=rscr2[qb * 8 : qb * 8 + 8, :], in_=rsb)

                def emit_norm(qb):
                    # normalization muls for all 4 (g,h) pairs of qb
                    for g in range(2):
                        for h in range(2):
                            stg = stg_tiles.pop((qb, g, h))
                            rcb = rcbp.tile([128, 512], BF16, tag="rcb",
                                            name=f"rcb_{qb}_{g}_{h}")
                            r0 = qb * 8 + g * 4 + h * 2
                            nc.sync.dma_start(
                                out=rcb[0:32, :],
                                in_=rscr2[r0 : r0 + 1, :].to_broadcast((32, 512)),
                            )
                            nc.sync.dma_start(
                                out=rcb[64:96, :],
                                in_=rscr2[r0 + 1 : r0 + 2, :].to_broadcast((32, 512)),
                            )
                            dst = ctxT_sb[2 * g + h]
                            nc.vector.tensor_mul(
                                out=dst[0:32, qb * 512 : qb * 512 + 512],
                                in0=stg[0:32, :], in1=rcb[0:32, :],
                            )
                            nc.vector.tensor_mul(
                                out=dst[64:96, qb * 512 : qb * 512 + 512],
                                in0=stg[64:96, :], in1=rcb[64:96, :],
                            )

                step = 0
                for qb in range(4):
                    for g in range(2):
                        ctx_t = [
                            cxp.tile([128, 512], F32, tag="cx",
                                     name=f"cx_{qb}_{g}_{h}")
                            for h in range(2)
                        ]
                        eTs_by_kt = {}
                        for kt in range(16):
                            # deferred normalization work for qb-1 (emitted
                            # here so its DMA roundtrip latency never blocks
                            # the in-order DVE exp stream)
                            if qb > 0 and g == 0:
                                if kt == 2:
                                    emit_recip(qb - 1)
                                if kt == 9:
                                    emit_norm(qb - 1)
                            eTs = []
                            for h in range(2):
                                sc = scp.tile([128, 1024], F32, tag="sc",
                                              name=f"sc_{qb}_{g}_{kt}_{h}")
                                for jj, j in enumerate((2 * h, 2 * h + 1)):
                                    nc.tensor.matmul(
                                        out=sc[:, jj * 512 : jj * 512 + 512],
                                        lhsT=kT_sb[32 * j : 32 * j + 32,
                                                   g * S + kt * 128 : g * S + kt * 128 + 128],
                                        rhs=qT_sb[32 * j : 32 * j + 32,
                                                  g * S + qb * 512 : g * S + qb * 512 + 512],
                                        start=True, stop=True,
                                        tile_position=(32 * j, 0),
                                    )
                                eT = etp.tile([128, 1024], BF16, tag="eT")
                                if ACT_PAT[step % 32]:
                                    nc.scalar.activation(
                                        out=eT, in_=sc, func=EXP, scale=SCALE,
                                    )
                                else:
                                    nc.vector.tensor_scalar(
                                        out=eT.bitcast(I16), in0=sc,
                                        scalar1=EXP_A, scalar2=EXP_B,
                                        op0=mybir.AluOpType.mult,
                                        op1=mybir.AluOpType.add,
                                    )
                                eTs.append(eT)
                                step += 1
                            eTs_by_kt[kt] = eTs
                            if kt >= 1:
                                emit_ctx(qb, g, kt - 1, ctx_t,
                                         eTs_by_kt.pop(kt - 1))
                        emit_ctx(qb, g, 15, ctx_t, eTs_by_kt.pop(15))
                        # evict ctx PSUM (bf16 staging, one [97,512] copy per
                        # pair); the rowsum rows ride along in bf16 and are
                        # DMA'd out for the batched reciprocal
                        for h in range(2):
                            stg = stgp.tile([128, 512], BF16, tag="stg",
                                            name=f"stg_{qb}_{g}_{h}")
                            nc.vector.tensor_copy(
                                out=stg[0:97, :], in_=ctx_t[h][0:97, :]
                            )
                            stg_tiles[(qb, g, h)] = stg
                            r0 = qb * 8 + g * 4 + h * 2
                            nc.sync.dma_start(out=rscr[r0 : r0 + 1, :],
                                              in_=stg[32:33, :])
                            nc.sync.dma_start(out=rscr[r0 + 1 : r0 + 2, :],
                                              in_=stg[96:97, :])
                # tail normalization for the last q-block
                emit_recip(3)
                emit_norm(3)

            # ---- phase 4: out = ctxT^T @ w_out_perm + b_out; evicted via
            #      ACT copies (idle in the tail) ----
            with tc.tile_pool(name="op", bufs=4, space="PSUM") as op:
                for st in range(16):
                    ps = op.tile([128, 256], F32, tag="op")
                    for kk in range(4):
                        nc.tensor.matmul(
                            out=ps,
                            lhsT=ctxT_sb[kk][:, st * 128 : st * 128 + 128],
                            rhs=wo_sb[:, kk * 256 : kk * 256 + 256],
                            start=(kk == 0), stop=(kk == 3),
                        )
                    ot = ev.tile([128, 256], F32, tag="ot")
                    nc.scalar.copy(out=ot, in_=ps)
                    nc.sync.dma_start(
                        out=out[st * 128 : st * 128 + 128, :], in_=ot
                    )
    if legalize:
        _legalize_sync_waits(nc)
    return nc


_NC_CACHE = None


def _get_nc():
    global _NC_CACHE
    if _NC_CACHE is None:
        _NC_CACHE = _build_nc()
    return _NC_CACHE


def _ks_layout(a, nk, cols):
    """[nk*128, cols] -> [128, nk*cols] with [p, k*cols+c] = a[k*128+p, c]."""
    return np.ascontiguousarray(
        a.reshape(nk, 128, cols).transpose(1, 0, 2).reshape(128, nk * cols)
    )


def _prep_in_maps(x, w_qkv, b_qkv, w_out, b_out):
    x = np.asarray(x, dtype=np.float32)
    w_qkv = np.asarray(w_qkv, dtype=np.float32)
    b_qkv = np.asarray(b_qkv, dtype=np.float32)
    w_out = np.asarray(w_out, dtype=np.float32)
    b_out = np.asarray(b_out, dtype=np.float32)

    # shared (per-core identical) weight layouts
    wqk_l = _ks_layout(w_qkv[:, : 2 * H], 2, 512).astype(NPBF16)

    # v weights: 64-wide slot per head: [v_h (32) | ones-col | 31 zero]
    # (the ones column itself is planted via the bias matmul; v bias is the
    # spec's b_qkv v-slice)
    wpad = np.zeros((H, 264), np.float32)
    bvr = np.zeros((1, 264), np.float32)
    for h in range(NH):
        c0 = h * 33
        wpad[:, c0 : c0 + 32] = w_qkv[:, 2 * H + h * HD : 2 * H + (h + 1) * HD]
        bvr[0, c0 : c0 + 32] = b_qkv[2 * H + h * HD : 2 * H + (h + 1) * HD]
        bvr[0, c0 + 32] = 1.0  # ones column -> rowsum row
    wv_l = _ks_layout(wpad, 2, 264).astype(NPBF16)

    # w_out rows permuted into the ctxT slot layout (zeros in pad slots)
    wo_perm = np.zeros((512, H), np.float32)
    for pair in range(4):
        for side in range(2):
            h = 2 * pair + side
            r0 = pair * 128 + side * 64
            wo_perm[r0 : r0 + 32, :] = w_out[h * HD : (h + 1) * HD, :]
    wo_perm[32, :] = b_out  # multiplied by the ctxT[0] ones row
    wo_l = _ks_layout(wo_perm, 4, 256).astype(NPBF16)

    shared = {
        "wqk": wqk_l,
        "wv": wv_l,
        "bv": bvr.astype(NPBF16),
        "wo": wo_l,
        "bqkc": np.ascontiguousarray(
            b_qkv[: 2 * H].astype(np.float32).reshape(4, 128).T
        ),
        "ones": np.ones((1, 512), NPBF16),
        "zrow": np.concatenate([np.zeros((1, 2048), NPBF16), np.ones((1, 2048), NPBF16)]),
    }
    in_maps = []
    for b in range(B):
        xt = _ks_layout(np.ascontiguousarray(x[b].T), 2, S).astype(NPBF16)
        in_maps.append({"xt": xt, **shared})
    return in_maps


def kernel(x, w_qkv, b_qkv, w_out, b_out):
    in_maps = _prep_in_maps(x, w_qkv, b_qkv, w_out, b_out)
    nc = _get_nc()
    res = run_bass_kernel_spmd(nc, in_maps, list(range(N_CORES)), **TRACE_OPTS)
    global LAST_RESULT
    LAST_RESULT = res
    return np.stack([res.results[b]["out"] for b in range(B)], axis=0)


# revision 13
# speedup vs baseline: 1.5632x; 1.0833x over previous
"""Multi-head self-attention (B=8, S=2048, H=256, NH=8, HD=32) on 8 TRN2 cores.

Strategy: data-parallel over batch — each core computes full MHA for one
batch element; no collectives.

Per-core dataflow (matmuls bf16 in / fp32 PSUM accum):
  - host ships x^T (features on partitions) so no on-device transpose
  - qkT:  q^T,k^T [feat, s] = w_qkv^T @ x — feature-major so each head's
    32 q/k features land on one 32-partition strip
  - attention runs qb(4) x g(2 head-groups) x kt(16) x half(2):
    per step 2 row-tiled scores matmuls (tile_position=(32j,0), K=32)
    into a 3-deep rotation of [128,1024] PSUM tiles, then ONE exp:
      - ACT steps: scalar ACTIVATE Exp (scale folded in)
      - DVE steps: Schraudolph bf16 exp — tensor_scalar mult+add to an
        int16 view of the bf16 eT tile (i16 = rne(s*A + B) IS the bf16
        bit pattern of ~exp(s*scale)); rowsum-normalization cancels the
        systematic part of the approx error per head
    Splitting exp across both engines breaks the single-engine ACT
    bottleneck (~331us of EXP in the previous version).
  - ctx^T accumulated over kt with 2x column-tiled PE (tile_position
    (0,0)/(0,64)); stationary v blocks carry a ones column so each
    64-wide head slot yields [ctx_h(32) | rowsum(1)]; ctx for kt-1 is
    emitted as a 4-matmul col-mode burst after kt's scores so row/col
    mode switches happen 2x per kt, mostly hidden behind exp waits
  - g-outer keeps only 2 ctx accumulator banks live -> scores get a
    3-tile rotation (6 banks) which decouples the scores->exp WAR chain
  - per (qb,g): ctx PSUM evicted to bf16 staging in one [97,512] copy;
    rowsum rows DMA'd straight from PSUM (fp32) to DRAM; per qb one
    batched fast-reciprocal + bf16 broadcast roundtrip via DRAM; the
    normalization multiplies (bf16, 2x/4x DVE modes) are emitted
    deferred (during the next qb) so DMA latency never stalls the DVE
  - out = ctxT^T @ w_out_perm + b_out as a tail phase; out-proj PSUM is
    DMA'd directly to DRAM (no SBUF staging)
"""
import numpy as np
import ml_dtypes

import bass_rust
import concourse.bass as bass
import concourse.mybir as mybir
import concourse.tile as tile
from concourse.bass_utils import run_bass_kernel_spmd

BF16 = mybir.dt.bfloat16
F32 = mybir.dt.float32
I16 = mybir.dt.int16
NPBF16 = ml_dtypes.bfloat16

B, S, H = 8, 2048, 256
NH, HD = 8, 32
SCALE = 1.0 / float(np.sqrt(HD))
N_CORES = 8

# Schraudolph bf16 exp constants: i16 = rne(s*A + B) viewed as bf16
# approximates exp(s*SCALE).  A = SCALE * 2^7 / ln2; B = 127*2^7 - c with
# c=1.5 calibrated for min global error (rne rounding confirmed on HW).
EXP_A = SCALE * 128.0 / float(np.log(2.0))
EXP_B = 16256.0 - 1.5

# ACT/DVE exp split: pattern over 32 steps, True -> ACT. 18/32 on ACT.
N_ACT_OF_32 = 18
ACT_PAT = [((i + 1) * N_ACT_OF_32) // 32 - (i * N_ACT_OF_32) // 32 == 1
           for i in range(32)]

# Set by a test harness to collect HW timing: {"trace": bool, "trace_cores": [...]}
TRACE_OPTS = {}
LAST_RESULT = None


def _legalize_sync_waits(nc):
    """The walrus build here rejects >1 sync wait per instruction, but Tile
    freely emits 2-3 (and the exit drain up to ~27).  Move excess waits onto
    same-engine NoOp carriers inserted immediately before the offending
    instruction — identical semantics (the engine blocks on each wait in
    program order)."""
    n = 0
    for f in nc.m.functions:
        for bb in f.blocks:
            insts = bb.instructions  # live list
            i = 0
            while i < len(insts):
                inst = insts[i]
                si = inst.sync_info
                if si is not None and len(si.on_wait) > 1:
                    waits = list(si.on_wait)
                    carriers = []
                    for w in waits[:-1]:
                        carriers.append(
                            mybir.InstNoOp(
                                name=f"{inst.name}-w{n}",
                                sync_info=mybir.SyncInfo(on_wait=[w], on_update=[]),
                                bass_nofuse=True,
                                engine=inst.engine,
                            )
                        )
                        n += 1
                    inst.sync_info = bass_rust.SyncInfo(
                        on_wait=waits[-1:], on_update=list(si.on_update)
                    )
                    insts[i:i] = carriers
                    i += len(carriers)
                i += 1
    return n


def _build_nc(legalize=True):
    nc = bass.Bass()
    xt = nc.dram_tensor("xt", [128, 2 * S], BF16, kind="ExternalInput")
    wqk = nc.dram_tensor("wqk", [128, 2 * 512], BF16, kind="ExternalInput")
    bv = nc.dram_tensor("bv", [1, 264], BF16, kind="ExternalInput")
    wv = nc.dram_tensor("wv", [128, 2 * 264], BF16, kind="ExternalInput")
    wo = nc.dram_tensor("wo", [128, 4 * 256], BF16, kind="ExternalInput")
    bqkc = nc.dram_tensor("bqkc", [128, 4], F32, kind="ExternalInput")
    ones = nc.dram_tensor("ones", [1, 512], BF16, kind="ExternalInput")
    zrow = nc.dram_tensor("zrow", [2, 2048], BF16, kind="ExternalInput")
    out = nc.dram_tensor("out", [S, H], F32, kind="ExternalOutput")
    # scratch for the rowsum reciprocal roundtrip (SBUF APs cannot have a
    # zero partition step, DRAM APs can): one row per (qb, g, h, side)
    rscr2 = nc.dram_tensor("rscr2", [32, 512], BF16)

    EXP = mybir.ActivationFunctionType.Exp

    with tile.TileContext(nc) as tc:
        with (
            tc.tile_pool(name="const", bufs=1) as const,
            tc.tile_pool(name="ev", bufs=6) as ev,
            tc.tile_pool(name="stgp", bufs=4) as stgp,
            tc.tile_pool(name="rcbp", bufs=4) as rcbp,
            tc.tile_pool(name="etp", bufs=6) as etp,
        ):
            xt_sb = const.tile([128, 2 * S], BF16, tag="xt")
            nc.sync.dma_start(out=xt_sb, in_=xt[:, :])
            wqk_sb = const.tile([128, 2 * 512], BF16, tag="wqk")
            nc.sync.dma_start(out=wqk_sb, in_=wqk[:, :])
            wv_sb = const.tile([128, 2 * 264], BF16, tag="wv")
            nc.sync.dma_start(out=wv_sb, in_=wv[:, :])
            wo_sb = const.tile([128, 4 * 256], BF16, tag="wo")
            nc.sync.dma_start(out=wo_sb, in_=wo[:, :])
            bv_sb = const.tile([1, 264], BF16, tag="bv")
            nc.sync.dma_start(out=bv_sb, in_=bv[:, :])
            ones1_sb = const.tile([1, 128], BF16, tag="ones1")
            nc.sync.dma_start(out=ones1_sb, in_=ones[0:1, 0:128])
            bqkc_sb = const.tile([128, 4], F32, tag="bqkc")
            nc.sync.dma_start(out=bqkc_sb, in_=bqkc[:, :])

            qT_sb = const.tile([128, 2 * S], BF16, tag="qT")
            kT_sb = const.tile([128, 2 * S], BF16, tag="kT")
            v_sb = const.tile([128, 16 * 264], BF16, tag="v")
            ctxT_sb = [
                const.tile([128, S], BF16, tag=f"ctxT{k}", name=f"ctxT{k}")
                for k in range(4)
            ]
            # rows 32:64 / 96:128 of each ctxT tile are never written by the
            # normalization muls but are contracted by the output matmul
            # (against zeroed w_out rows) — clear them via broadcast DMA so
            # stale NaN patterns can't poison the accumulation
            for k in range(4):
                if k == 0:
                    # row 32 of tile 0 is all-ones: paired with w_out_perm
                    # row 32 = b_out it adds the output bias for free
                    nc.sync.dma_start(out=ctxT_sb[0][32:33, :], in_=zrow[1:2, :])
                    nc.sync.dma_start(
                        out=ctxT_sb[0][33:64, :],
                        in_=zrow[0:1, :].to_broadcast((31, S)),
                    )
                else:
                    nc.sync.dma_start(
                        out=ctxT_sb[k][32:64, :],
                        in_=zrow[0:1, :].to_broadcast((32, S)),
                    )
                nc.sync.dma_start(
                    out=ctxT_sb[k][96:128, :],
                    in_=zrow[0:1, :].to_broadcast((32, S)),
                )

            # ---- phase 0: HAM warmup — dep-free back-to-back matmuls so the
            # PE clock gate opens (1.2 -> 2.4 GHz) before the real work ----
            with tc.tile_pool(name="pp", bufs=4, space="PSUM") as pp:
                warm_sb = const.tile([128, 512], BF16, tag="warm")
                nc.vector.memset(warm_sb, 0.0)
                warm_ps = pp.tile([128, 512], F32, tag="pp")
                for _ in range(6):
                    nc.tensor.matmul(
                        out=warm_ps, lhsT=warm_sb[:, 0:128], rhs=warm_sb[:, :],
                        start=True, stop=True,
                    )

                # ---- phase 1: qT/kT [feature, s] = w_qkv^T @ x; bias folded
                #      into the eviction (per-partition, features-major) ----
                for t in range(4):  # feature tiles: q0,q1,k0,k1
                    for nb in range(4):  # s blocks of 512
                        ps = pp.tile([128, 512], F32, tag="pp")
                        for ks in range(2):
                            nc.tensor.matmul(
                                out=ps,
                                lhsT=wqk_sb[:, ks * 512 + t * 128 : ks * 512 + t * 128 + 128],
                                rhs=xt_sb[:, ks * S + nb * 512 : ks * S + nb * 512 + 512],
                                start=(ks == 0), stop=(ks == 1),
                            )
                        dst = (qT_sb if t < 2 else kT_sb)[
                            :, (t % 2) * S + nb * 512 : (t % 2) * S + nb * 512 + 512
                        ]
                        nc.vector.tensor_scalar_add(
                            out=dst, in0=ps, scalar1=bqkc_sb[:, t : t + 1]
                        )

                # ---- phase 2: v (natural layout, padded 64-wide head slots,
                #      ones column at j=32 for rowsums) ----
                for st in range(16):
                    ps = pp.tile([128, 264], F32, tag="ppv")
                    for ks in range(2):
                        nc.tensor.matmul(
                            out=ps,
                            lhsT=xt_sb[:, ks * S + st * 128 : ks * S + st * 128 + 128],
                            rhs=wv_sb[:, ks * 264 : ks * 264 + 264],
                            start=(ks == 0), stop=False,
                        )
                    # bias row also plants the rowsum ones-columns
                    nc.tensor.matmul(
                        out=ps,
                        lhsT=ones1_sb[0:1, 0:128],
                        rhs=bv_sb[0:1, 0:264],
                        start=False, stop=True,
                    )
                    dst = v_sb[:, st * 264 : st * 264 + 264]
                    nc.scalar.copy(out=dst, in_=ps)

            # ---- phase 3: attention ----
            stg_tiles = {}   # (qb, g, h) -> stg tile
            rsg_tiles = {}   # (qb, g) -> packed bf16 rowsum gather tile

            with (
                tc.tile_pool(name="scp", bufs=3, space="PSUM") as scp,
                tc.tile_pool(name="cxp", bufs=2, space="PSUM") as cxp,
            ):
                def emit_ctx(qb, g, kt, ctx_t, eTs):
                    # ctx accumulation for (g, kt): 4 col-mode matmuls
                    for h in range(2):
                        cps = ctx_t[h]
                        eT = eTs[h]
                        vc = kt * 264 + (4 * g + 2 * h) * 33
                        nc.tensor.matmul(
                            out=cps[0:33, :],
                            lhsT=v_sb[:, vc : vc + 33],
                            rhs=eT[:, 0:512],
                            start=(kt == 0), stop=(kt == 15),
                            tile_position=(0, 0), skip_group_check=True,
                        )
                        nc.tensor.matmul(
                            out=cps[64:97, :],
                            lhsT=v_sb[:, vc + 33 : vc + 66],
                            rhs=eT[:, 512:1024],
                            start=(kt == 0), stop=(kt == 15),
                            tile_position=(0, 64), skip_group_check=True,
                        )

                def emit_recip(qb, g):
                    # rowsum reciprocal for the 4 (h,side) rows of (qb,g),
                    # packed [64,32] (free size 32) so the DVE cost is tiny;
                    # result lands in rscr2 rows for the broadcast DMAs
                    rsgb = rsg_tiles.pop((qb, g))
                    rsgf = ev.tile([64, 32], F32, tag="rsgf")
                    nc.vector.tensor_copy(out=rsgf, in_=rsgb)
                    rsr = ev.tile([64, 32], F32, tag="rsr")
                    nc.vector.reciprocal(out=rsr, in_=rsgf)
                    rsb = ev.tile([64, 32], BF16, tag="rsb")
                    nc.vector.tensor_copy(out=rsb, in_=rsr)
                    r0 = qb * 8 + g * 4
                    nc.sync.dma_start(out=rscr2[r0 : r0 + 4, :], in_=rsb)

                def emit_norm(qb, g):
                    # normalization muls for the 2 (h) pairs of (qb,g)
                    for h in range(2):
                        stg = stg_tiles.pop((qb, g, h))
                        rcb = rcbp.tile([128, 512], BF16, tag="rcb",
                                        name=f"rcb_{qb}_{g}_{h}")
                        r0 = qb * 8 + g * 4 + h * 2
                        nc.sync.dma_start(
                            out=rcb[0:32, :],
                            in_=rscr2[r0 : r0 + 1, :].to_broadcast((32, 512)),
                        )
                        nc.sync.dma_start(
                            out=rcb[64:96, :],
                            in_=rscr2[r0 + 1 : r0 + 2, :].to_broadcast((32, 512)),
                        )
                        dst = ctxT_sb[2 * g + h]
                        nc.vector.tensor_mul(
                            out=dst[0:32, qb * 512 : qb * 512 + 512],
                            in0=stg[0:32, :], in1=rcb[0:32, :],
                        )
                        nc.vector.tensor_mul(
                            out=dst[64:96, qb * 512 : qb * 512 + 512],
                            in0=stg[64:96, :], in1=rcb[64:96, :],
                        )

                step = 0
                groups = [(qb, g) for qb in range(4) for g in range(2)]
                for gi, (qb, g) in enumerate(groups):
                    ctx_t = [
                        cxp.tile([128, 512], F32, tag="cx",
                                 name=f"cx_{qb}_{g}_{h}")
                        for h in range(2)
                    ]
                    eTs_by_kt = {}
                    for kt in range(16):
                        # deferred normalization work for the previous group
                        # (emitted here so its DMA roundtrip latency never
                        # blocks the in-order DVE exp stream)
                        if gi > 0:
                            if kt == 3:
                                emit_recip(*groups[gi - 1])
                            if kt == 9:
                                emit_norm(*groups[gi - 1])
                        # scores: all 4 row-tiled matmuls of this kt adjacent
                        # so they overlap 4-way on the PE
                        scs = []
                        for h in range(2):
                            sc = scp.tile([128, 1024], F32, tag="sc",
                                          name=f"sc_{qb}_{g}_{kt}_{h}")
                            for jj, j in enumerate((2 * h, 2 * h + 1)):
                                nc.tensor.matmul(
                                    out=sc[:, jj * 512 : jj * 512 + 512],
                                    lhsT=kT_sb[32 * j : 32 * j + 32,
                                               g * S + kt * 128 : g * S + kt * 128 + 128],
                                    rhs=qT_sb[32 * j : 32 * j + 32,
                                              g * S + qb * 512 : g * S + qb * 512 + 512],
                                    start=True, stop=True,
                                    tile_position=(32 * j, 0),
                                )
                            scs.append(sc)
                        eTs = []
                        for h in range(2):
                            eT = etp.tile([128, 1024], BF16, tag="eT")
                            if ACT_PAT[step % 32]:
                                nc.scalar.activation(
                                    out=eT, in_=scs[h], func=EXP, scale=SCALE,
                                )
                            else:
                                nc.vector.tensor_scalar(
                                    out=eT.bitcast(I16), in0=scs[h],
                                    scalar1=EXP_A, scalar2=EXP_B,
                                    op0=mybir.AluOpType.mult,
                                    op1=mybir.AluOpType.add,
                                )
                            eTs.append(eT)
                            step += 1
                        eTs_by_kt[kt] = eTs
                        if kt >= 1:
                            emit_ctx(qb, g, kt - 1, ctx_t,
                                     eTs_by_kt.pop(kt - 1))
                    emit_ctx(qb, g, 15, ctx_t, eTs_by_kt.pop(15))
                    # evict ctx PSUM (bf16 staging, one [97,512] copy per
                    # pair); rowsum rows (32/96, in bf16) are gathered via
                    # tiny SBUF->SBUF DMAs into a packed [64,32] tile for
                    # the reciprocal (row j of 512 -> 16 partitions x 32)
                    rsgb = ev.tile([64, 32], BF16, tag="rsgb",
                                   name=f"rsgb_{qb}_{g}")
                    for h in range(2):
                        stg = stgp.tile([128, 512], BF16, tag="stg",
                                        name=f"stg_{qb}_{g}_{h}")
                        nc.vector.tensor_copy(
                            out=stg[0:97, :], in_=ctx_t[h][0:97, :]
                        )
                        stg_tiles[(qb, g, h)] = stg
                        p0 = h * 32
                        nc.sync.dma_start(out=rsgb[p0 : p0 + 16, :],
                                          in_=stg[32:33, :])
                        nc.sync.dma_start(out=rsgb[p0 + 16 : p0 + 32, :],
                                          in_=stg[96:97, :])
                    rsg_tiles[(qb, g)] = rsgb
                # tail normalization for the last group
                emit_recip(3, 1)
                emit_norm(3, 1)

            # ---- phase 4: out = ctxT^T @ w_out_perm + b_out; evicted via
            #      ACT copies (idle in the tail) ----
            with tc.tile_pool(name="op", bufs=4, space="PSUM") as op:
                for st in range(16):
                    ps = op.tile([128, 256], F32, tag="op")
                    for kk in range(4):
                        nc.tensor.matmul(
                            out=ps,
                            lhsT=ctxT_sb[kk][:, st * 128 : st * 128 + 128],
                            rhs=wo_sb[:, kk * 256 : kk * 256 + 256],
                            start=(kk == 0), stop=(kk == 3),
                        )
                    ot = ev.tile([128, 256], F32, tag="ot")
                    if st % 2 == 0:
                        nc.scalar.copy(out=ot, in_=ps)
                    else:
                        nc.vector.tensor_copy(out=ot, in_=ps)
                    nc.sync.dma_start(
                        out=out[st * 128 : st * 128 + 128, :], in_=ot
                    )
    if legalize:
        _legalize_sync_waits(nc)
    return nc


_NC_CACHE = None


def _get_nc():
    global _NC_CACHE
    if _NC_CACHE is None:
        _NC_CACHE = _build_nc()
    return _NC_CACHE


def _ks_layout(a, nk, cols):
    """[nk*128, cols] -> [128, nk*cols] with [p, k*cols+c] = a[k*128+p, c]."""
    return np.ascontiguousarray(
        a.reshape(nk, 128, cols).transpose(1, 0, 2).reshape(128, nk * cols)
    )


def _prep_in_maps(x, w_qkv, b_qkv, w_out, b_out):
    x = np.asarray(x, dtype=np.float32)
    w_qkv = np.asarray(w_qkv, dtype=np.float32)
    b_qkv = np.asarray(b_qkv, dtype=np.float32)
    w_out = np.asarray(w_out, dtype=np.float32)
    b_out = np.asarray(b_out, dtype=np.float32)

    # shared (per-core identical) weight layouts
    wqk_l = _ks_layout(w_qkv[:, : 2 * H], 2, 512).astype(NPBF16)

    # v weights: 64-wide slot per head: [v_h (32) | ones-col | 31 zero]
    # (the ones column itself is planted via the bias matmul; v bias is the
    # spec's b_qkv v-slice)
    wpad = np.zeros((H, 264), np.float32)
    bvr = np.zeros((1, 264), np.float32)
    for h in range(NH):
        c0 = h * 33
        wpad[:, c0 : c0 + 32] = w_qkv[:, 2 * H + h * HD : 2 * H + (h + 1) * HD]
        bvr[0, c0 : c0 + 32] = b_qkv[2 * H + h * HD : 2 * H + (h + 1) * HD]
        bvr[0, c0 + 32] = 1.0  # ones column -> rowsum row
    wv_l = _ks_layout(wpad, 2, 264).astype(NPBF16)

    # w_out rows permuted into the ctxT slot layout (zeros in pad slots)
    wo_perm = np.zeros((512, H), np.float32)
    for pair in range(4):
        for side in range(2):
            h = 2 * pair + side
            r0 = pair * 128 + side * 64
            wo_perm[r0 : r0 + 32, :] = w_out[h * HD : (h + 1) * HD, :]
    wo_perm[32, :] = b_out  # multiplied by the ctxT[0] ones row
    wo_l = _ks_layout(wo_perm, 4, 256).astype(NPBF16)

    shared = {
        "wqk": wqk_l,
        "wv": wv_l,
        "bv": bvr.astype(NPBF16),
        "wo": wo_l,
        "bqkc": np.ascontiguousarray(
            b_qkv[: 2 * H].astype(np.float32).reshape(4, 128).T
        ),
        "ones": np.ones((1, 512), NPBF16),
        "zrow": np.concatenate([np.zeros((1, 2048), NPBF16), np.ones((1, 2048), NPBF16)]),
    }
    in_maps = []
    for b in range(B):
        xt = _ks_layout(np.ascontiguousarray(x[b].T), 2, S).astype(NPBF16)
        in_maps.append({"xt": xt, **shared})
    return in_maps


def kernel(x, w_qkv, b_qkv, w_out, b_out):
    in_maps = _prep_in_maps(x, w_qkv, b_qkv, w_out, b_out)
    nc = _get_nc()
    res = run_bass_kernel_spmd(nc, in_maps, list(range(N_CORES)), **TRACE_OPTS)
    global LAST_RESULT
    LAST_RESULT = res
    return np.stack([res.results[b]["out"] for b in range(B)], axis=0)
